# revision 77
# baseline (speedup 1.0000x reference)
"""Trainium2 Bass kernel for causal multi-head attention (B=2, T=2048, C=1024, H=16, D=64).

Sharding (8 NeuronCores): data-parallel over the 2 batches x tensor-parallel over
4 head-groups (4 heads each). Core c handles batch c//4, heads 4*(c%4)..4*(c%4)+3.
Each core computes its 4 heads' QKV projections, causal attention, and a partial
output projection against its slice of Wo's columns; the host sums the 4 partials
per batch (the row-parallel unshard).

Schedule: one fused pipeline that keeps the PE stream dense (every stall
drops the PE out of its ramped clock and hardware activity-throttling caps
utilization, so idle gaps are the enemy):
  - all inputs host-packed partition-major (multi-KB DMA descriptor runs);
    critical-path data (m0 weight halves, x q-tile 0) dispatched first
    across the sync/scalar/gpsimd queues.
  - head-pair 0's attention row i starts right after x-chunk i's q/k/V
    projections; the m=1 q/k projections and the output projection are
    emitted as filler granules between attention units so the PE never
    waits on the ACT-engine exps that pace the softmax.
  - the AV matmul for tile j is emitted one unit late (its exp has already
    completed when the PE reaches it), and each hk1 row's out-proj tiles
    are popped in the following row after the deferred normalize flush.
  - softmax 1/l via ACT ln/exp on both halves' l strips (gathered to
    lanes 0/32 by DVE partition-shift copies straight from the AV PSUM
    ones-column -- no DMA hops in the chain).
  - y is written bf16 (host sums partials in fp32), per-tile, spread
    through the second half; y DMAs ride the gpsimd queue.
"""

from collections import deque

import numpy as np

B, T, C = 2, 2048, 1024
H, D = 16, 64
HPC = 4  # heads per core
N_CORES = 8
DH = HPC * D  # 256: per-core projection width

_compiled = None


def _patch_act_tables():
    """Make Exp resolve to one stable table set so softmax exps never thrash
    ACT_TABLE_LOADs."""
    import functools

    import concourse.hw_specs as hw_specs
    import concourse.mybir as mybir
    from concourse import bacc

    if getattr(bacc, "_act_tables_patched", False):
        return
    orig = hw_specs.get_activation_tables

    @functools.cache
    def patched(arch):
        tabs = {k: set(v) for k, v in orig(arch).items()}
        E = mybir.ActivationFunctionType.Exp
        L = mybir.ActivationFunctionType.Ln
        keep = "natural_log_exp_and_others"
        if keep in tabs and E in tabs[keep] and L in tabs[keep]:
            for name, fns in tabs.items():
                if name != keep:
                    fns.discard(E)
                    fns.discard(L)
        return tabs

    bacc.get_activation_tables = patched
    bacc._act_tables_patched = True


def _build():
    import concourse.bass as bass
    import concourse.mybir as mybir
    from concourse import bacc
    from concourse.tile import TileContext

    _patch_act_tables()

    dt = mybir.dt
    BF = dt.bfloat16
    F32 = dt.float32
    ts = bass.ts
    Act = mybir.ActivationFunctionType

    P = 128
    NQ = T // 512   # 4 q-tiles of 512
    NK = T // 128   # 16 key-tiles of 128
    KC = C // 128   # 8 contraction subtiles for the projections

    nc = bacc.Bacc("TRN2", target_bir_lowering=False, debug=False)

    # All inputs are host-packed partition-major so every DMA moves
    # multi-KB contiguous runs per partition (fast SWDGE descriptors).
    x_d = nc.dram_tensor("xP", [P, NQ, KC, 512], BF, kind="ExternalInput")
    wq_d = nc.dram_tensor("wqP", [P, DH // P, KC, P], BF, kind="ExternalInput")
    wk_d = nc.dram_tensor("wkP", [P, DH // P, KC, P], BF, kind="ExternalInput")
    wv_d = nc.dram_tensor("wvP", [P, KC, DH], BF, kind="ExternalInput")
    wo_d = nc.dram_tensor("woP", [P, DH // P, C], BF, kind="ExternalInput")
    y_d = nc.dram_tensor("y", [T, C], BF, kind="ExternalOutput")

    with TileContext(nc) as tc:
        with (
            tc.tile_pool(name="persist", bufs=1) as persist,
            tc.tile_pool(name="ptiles", bufs=8) as ptiles,
            tc.tile_pool(name="ytiles", bufs=4) as ytiles,
            tc.tile_pool(name="ltmp", bufs=2) as ltmp,
            tc.tile_pool(name="psum_s", bufs=2, space="PSUM") as psum_s,
            tc.tile_pool(name="psum_o", bufs=2, space="PSUM") as psum_o,
            tc.tile_pool(name="psum_p", bufs=2, space="PSUM") as psum_p,
        ):
            # ---- persistent SBUF tensors -------------------------------
            x_sb = persist.tile([P, KC, T], BF, tag="x")          # x^T
            wq_sb = persist.tile([P, DH // P, KC, P], BF, tag="wq")
            wk_sb = persist.tile([P, DH // P, KC, P], BF, tag="wk")
            wv_sb = persist.tile([P, KC, DH], BF, tag="wv")
            wo_sb = persist.tile([P, DH // P, C], BF, tag="wo")
            qT_sb = persist.tile([P, DH // P, T], BF, tag="qT")
            kT_sb = persist.tile([P, DH // P, T], BF, tag="kT")
            v_sb = persist.tile([P, NK, HPC, 66], BF, tag="v")
            oT_sb = persist.tile([P, DH // P, T], BF, tag="oT")
            cmask = persist.tile([P, 2, P], BF, tag="cmask")
            ytmp_sb = persist.tile([P, NK, C], BF, tag="ytmp")  # kc=0 partials

            # ---- DMA dispatch ------------------------------------------
            # weights fan out across four otherwise-idle queues so their
            # transfers overlap; x arrives q-tile-major (chunk n feeds the
            # n-th projection + attention row), ko-halves split between the
            # sync and scalar queues.
            # critical-path data first: m0 weight halves + x n0; the m1
            # weight halves, x n1-n3, and wo follow (needed 10-50us in)
            nc.gpsimd.dma_start(wq_sb[:, 0, 0:4], wq_d[:, 0, 0:4])
            nc.sync.dma_start(wk_sb[:, 0, 0:4], wk_d[:, 0, 0:4])
            # n=0 in ko-pair chunks so the first projection's ko-ascending
            # matmul chain starts on the first 128KB
            for kop in range(2):
                nc.sync.dma_start(
                    x_sb[:, 2 * kop : 2 * kop + 2, 0:512],
                    x_d[:, 0, 2 * kop : 2 * kop + 2, :],
                )
            nc.gpsimd.dma_start(wq_sb[:, 0, 4:8], wq_d[:, 0, 4:8])
            nc.sync.dma_start(wk_sb[:, 0, 4:8], wk_d[:, 0, 4:8])
            for kop in range(2, 4):
                nc.scalar.dma_start(
                    x_sb[:, 2 * kop : 2 * kop + 2, 0:512],
                    x_d[:, 0, 2 * kop : 2 * kop + 2, :],
                )
            nc.scalar.dma_start(wv_sb[:], wv_d[:])
            nc.gpsimd.dma_start(wq_sb[:, 1], wq_d[:, 1])
            nc.sync.dma_start(wk_sb[:, 1], wk_d[:, 1])
            for n in range(1, NQ):
                nc.sync.dma_start(
                    x_sb[:, 0:4, ts(n, 512)], x_d[:, n, 0:4, :]
                )
                nc.scalar.dma_start(
                    x_sb[:, 4:8, ts(n, 512)], x_d[:, n, 4:8, :]
                )
            nc.sync.dma_start(wo_sb[:], wo_d[:])

            # ---- pool queue: constants ---------------------------------
            nc.gpsimd.memset(v_sb[:, :, :, 64:66], 1.0)
            # diagonal [128,128] corner: keep 1.0 where col >= row, else 0.0
            nc.gpsimd.memset(cmask[:], 1.0)
            for half in range(2):
                nc.gpsimd.affine_select(
                    out=cmask[:, half, :],
                    in_=cmask[:, half, :],
                    compare_op=mybir.AluOpType.is_ge,
                    fill=0.0,
                    base=0,
                    pattern=[[1, P]],
                    channel_multiplier=-1,
                )

            # ---- filler machinery --------------------------------------
            # Generators yield after ~2 matmuls of emission; attention units
            # pop them to fill PE time while ACT exps run.
            fillers = deque()

            def pop_fillers(k):
                while k > 0 and fillers:
                    try:
                        next(fillers[0])
                        k -= 1
                    except StopIteration:
                        fillers.popleft()

            def drain_fillers():
                while fillers:
                    try:
                        next(fillers[0])
                    except StopIteration:
                        fillers.popleft()

            def qk_proj_gen(m, n):
                """Project q and k for dh-block m, q-tile n (via pp pool)."""
                pq = psum_p.tile([P, 512], F32, tag="pp")
                pk2 = psum_p.tile([P, 512], F32, tag="pp")
                for ko in range(KC):
                    nc.tensor.matmul(
                        pq[:], wq_sb[:, m, ko, :], x_sb[:, ko, ts(n, 512)],
                        start=(ko == 0), stop=(ko == KC - 1),
                    )
                    nc.tensor.matmul(
                        pk2[:], wk_sb[:, m, ko, :], x_sb[:, ko, ts(n, 512)],
                        start=(ko == 0), stop=(ko == KC - 1),
                    )
                    yield
                nc.vector.tensor_copy(qT_sb[:, m, ts(n, 512)], pq[:])
                nc.vector.tensor_copy(kT_sb[:, m, ts(n, 512)], pk2[:])

            def v_proj_gen(mt):
                """Project V for key-tile mt: out v_sb[t-tile, head, d]."""
                pv = psum_p.tile([P, 512], F32, tag="pp")
                for ko in range(KC):
                    nc.tensor.matmul(
                        pv[:, 0:DH], x_sb[:, ko, ts(mt, P)], wv_sb[:, ko, :],
                        start=(ko == 0), stop=(ko == KC - 1),
                    )
                    if ko % 2 == 1:
                        yield
                nc.vector.tensor_copy(
                    v_sb[:, mt, :, 0:64], pv[:, 0:DH].rearrange("p (h d) -> p h d", d=64)
                )

            def u1_chunk_gens(n):
                gens = [qk_proj_gen(0, n)]
                gens += [v_proj_gen(mt) for mt in range(4 * n, 4 * n + 4)]
                return gens

            def out_proj_kc0_gen(mt):
                """Y tile mt, hk0 half: needs only hk0's oT -> usable as
                filler from the very start of the hk1 phase."""
                py0 = psum_p.tile([P, 512], F32, tag="pp")
                py1 = psum_p.tile([P, 512], F32, tag="pp")
                pys = [py0, py1]
                for nn in range(C // 512):
                    nc.tensor.matmul(
                        pys[nn][:],
                        oT_sb[:, 0, ts(mt, P)],
                        wo_sb[:, 0, ts(nn, 512)],
                        start=True, stop=True,
                    )
                yield
                for nn in range(C // 512):
                    nc.vector.tensor_copy(
                        ytmp_sb[:, mt, ts(nn, 512)], pys[nn][:]
                    )

            def out_proj_kc1_gen(mt):
                """Y tile mt, hk1 half + combine with the stored kc0 partial
                (bf16 partial rounding stays well inside the error budget)."""
                py0 = psum_p.tile([P, 512], F32, tag="pp")
                py1 = psum_p.tile([P, 512], F32, tag="pp")
                pys = [py0, py1]
                for nn in range(C // 512):
                    nc.tensor.matmul(
                        pys[nn][:],
                        oT_sb[:, 1, ts(mt, P)],
                        wo_sb[:, 1, ts(nn, 512)],
                        start=True, stop=True,
                    )
                yield
                yt = ytiles.tile([P, C], BF, tag="y")
                for nn in range(C // 512):
                    nc.vector.tensor_add(
                        yt[:, ts(nn, 512)],
                        ytmp_sb[:, mt, ts(nn, 512)],
                        pys[nn][:],
                    )
                nc.gpsimd.dma_start(y_d[ts(mt, P), :], yt[:])

            # ---- softmax normalization (baseline-proven chain) ---------
            # normalize multiplies are deferred one attention unit so the
            # DVE never head-of-line blocks on the recip/broadcast chain
            pending_mults = []

            def flush_mults():
                for args in pending_mults:
                    nc.vector.tensor_mul(*args)
                pending_mults.clear()

            def emit_recip(hk, i, l01):
                # l01: [64,512] with l strips for the halves at lanes 0, 32
                # (partition-shift ops need multiple-of-32 bases)
                lnl = ltmp.tile([64, 512], F32, tag="lnl")
                nc.scalar.activation(lnl[:], l01[:], Act.Ln)
                rec2 = ltmp.tile([64, 512], BF, tag="rec2")
                nc.scalar.activation(rec2[:], lnl[:], Act.Exp, scale=-1.0)
                for half in range(2):
                    if half == 0:
                        src = rec2[0:1, :]
                    else:
                        # broadcast ucode reads true lane 0 only; DVE
                        # partition-shift copy (proven) beats a DMA bounce
                        rec1 = ltmp.tile([1, 512], BF, tag="rec1")
                        nc.vector.tensor_copy(rec1[0:1, :], rec2[32:33, :])
                        src = rec1[0:1, :]
                    rb = ltmp.tile([P, 512], BF, tag="rb")
                    nc.gpsimd.partition_broadcast(rb[:], src)
                    hp = 64 * half
                    pending_mults.append(
                        (
                            oT_sb[hp : hp + 64, hk, ts(i, 512)],
                            oT_sb[hp : hp + 64, hk, ts(i, 512)],
                            rb[hp : hp + 64, :],
                        )
                    )

            # ---- attention unit: one (hk, i) q-tile row ---------------
            # prev row's deferred normalize mults flush at unit 2 (their rb
            # broadcast has resolved by then); out-proj pops start at unit
            # >= 2 so their conservative oT-write dependency (everything
            # emitted before them, incl. that flush) is already resolved
            def attn_i(hk, i, pops_per_unit, pop_start=0):
                jmax = 4 * i + 3
                sps = {}

                def emit_S(j):
                    c0 = P * (j - 4 * i) if j >= 4 * i else 0
                    sp = psum_s.tile([P, 2, 512], F32, tag="s")
                    # row-group-packed pair: head 2*hk in PE rows 0-63,
                    # head 2*hk+1 in rows 64-127 -> concurrent matmuls
                    for half in range(2):
                        hp = 64 * half
                        nc.tensor.matmul(
                            sp[:, half, c0:],
                            kT_sb[hp : hp + 64, hk, ts(j, P)],
                            qT_sb[hp : hp + 64, hk, 512 * i + c0 : 512 * (i + 1)],
                            start=True,
                            stop=True,
                            tile_position=(hp, 0),
                        )
                    sps[j] = sp

                op0 = psum_o.tile([P, 512], F32, tag="o")
                op1 = psum_o.tile([P, 512], F32, tag="o")
                ops = [op0, op1]

                def emit_AV(j, pt):
                    c0 = P * (j - 4 * i) if j >= 4 * i else 0
                    for half in range(2):
                        h = 2 * hk + half
                        nc.tensor.matmul(
                            ops[half][0:65, c0:],
                            v_sb[:, j, h, 0:65],
                            pt[:, half, c0:],
                            start=(j == 0),
                            stop=(j == jmax),
                        )

                emit_S(0)
                pending_av = None
                for j in range(jmax + 1):
                    if j == min(2, jmax):
                        flush_mults()  # prev row's deferred normalizes
                    if j + 1 <= jmax:
                        emit_S(j + 1)
                    c0 = P * (j - 4 * i) if j >= 4 * i else 0
                    sp = sps.pop(j)
                    pt = ptiles.tile([P, 2, 512], BF, tag="p")
                    if j >= 4 * i:
                        t = j - 4 * i
                        # cols < 128t are never computed nor read
                        nc.scalar.activation(
                            pt[:, :, P * t :], sp[:, :, P * t :], Act.Exp, scale=0.125
                        )
                        nc.vector.tensor_mul(
                            pt[:, :, P * t : P * (t + 1)],
                            pt[:, :, P * t : P * (t + 1)],
                            cmask[:],
                        )
                    else:
                        nc.scalar.activation(pt[:], sp[:], Act.Exp, scale=0.125)
                    if j >= pop_start:
                        pop_fillers(pops_per_unit)
                    # AV lags one unit: when the PE reaches it, its exp has
                    # long completed, so the PE stream never blocks here
                    if pending_av is not None:
                        emit_AV(*pending_av)
                    pending_av = (j, pt)
                emit_AV(*pending_av)
                # finalize: copy O^T out, gather both l strips (PSUM lane
                # 64 -> SBUF lanes 0/1 via partition-shift copies), recip
                l01 = ltmp.tile([64, 512], F32, tag="lh")
                for half in range(2):
                    hp = 64 * half
                    nc.vector.tensor_copy(
                        oT_sb[hp : hp + 64, hk, ts(i, 512)], ops[half][0:64, :]
                    )
                    nc.vector.tensor_copy(
                        l01[32 * half : 32 * half + 1, :], ops[half][64:65, :]
                    )
                emit_recip(hk, i, l01)

            # ---- fused pipeline ---------------------------------------
            # chunk 0: only q/k inline; V0..V3 are the first fillers (V0/V1
            # fully emit in unit 0, one unit before the lagged AV needs them)
            for _ in qk_proj_gen(0, 0):
                pass
            for mt in range(4):
                fillers.append(v_proj_gen(mt))
            for i in range(NQ):
                if i < 3:
                    fillers.extend(u1_chunk_gens(i + 1))
                else:
                    # n=3 first: attn(1,3) runs right after attn(0,3)
                    for n in (3, 0, 1, 2):
                        fillers.append(qk_proj_gen(1, n))
                attn_i(0, i, pops_per_unit=(8, 5, 3, 2)[i])
            # hk=1 rows: the kc=0 out-proj halves (hk0-only deps) fill row
            # (1,3)'s large PE deficit; each row's kc=1 halves pop in the
            # next row once the mid-row flush has resolved their mults.
            # Tiles 12-15's kc0 pops land after (1,3)'s unit-2 flush of
            # the (0,3) normalize mults they read.
            for mt in range(NK):
                fillers.append(out_proj_kc0_gen(mt))
            attn_i(1, 3, pops_per_unit=2, pop_start=1)
            for mt in range(12, 16):
                fillers.append(out_proj_kc1_gen(mt))
            attn_i(1, 1, pops_per_unit=2, pop_start=2)
            for mt in range(4, 8):
                fillers.append(out_proj_kc1_gen(mt))
            attn_i(1, 2, pops_per_unit=2, pop_start=2)
            for mt in range(8, 12):
                fillers.append(out_proj_kc1_gen(mt))
            attn_i(1, 0, pops_per_unit=3, pop_start=2)
            drain_fillers()
            flush_mults()
            for mt in range(0, 4):
                for _ in out_proj_kc1_gen(mt):
                    pass

    nc.compile()
    return nc


def _get_compiled():
    global _compiled
    if _compiled is None:
        _compiled = _build()
    return _compiled


def make_inputs(x, Wq, Wk, Wv, Wo):
    """Shard the full inputs into the 8 per-core input maps (host-side prep).

    Everything is packed partition-major ([128, ...] with large contiguous
    per-partition runs) so device DMAs use few, large descriptors."""
    import ml_dtypes

    bf16 = ml_dtypes.bfloat16
    x = np.asarray(x)

    def pack_w(wT):  # [C, DH] -> [128, 2, KC, 128] (m-major)
        return np.ascontiguousarray(
            wT.reshape(C // 128, 128, DH // 128, 128).transpose(1, 2, 0, 3)
        ).astype(bf16)

    def pack_wv(wT):  # [C, DH] -> [128, KC, DH]
        return np.ascontiguousarray(
            wT.reshape(C // 128, 128, DH).transpose(1, 0, 2)
        ).astype(bf16)

    in_maps = []
    for c in range(N_CORES):
        b, g = divmod(c, HPC)
        rows = slice(g * DH, (g + 1) * DH)
        xT = x[b].T  # [C, T]
        xP = np.ascontiguousarray(
            xT.reshape(C // 128, 128, T // 512, 512).transpose(1, 2, 0, 3)
        ).astype(bf16)  # [128, NQ, KC, 512]
        woT = np.asarray(Wo)[:, rows].T  # [DH, C]
        woP = np.ascontiguousarray(
            woT.reshape(DH // 128, 128, C).transpose(1, 0, 2)
        ).astype(bf16)  # [128, 2, C]
        in_maps.append(
            {
                "xP": xP,
                "wqP": pack_w(np.asarray(Wq)[rows, :].T),
                "wkP": pack_w(np.asarray(Wk)[rows, :].T),
                "wvP": pack_wv(np.asarray(Wv)[rows, :].T),
                "woP": woP,
            }
        )
    return in_maps


def assemble(results):
    """Sum the 4 tensor-parallel partials per batch into the full output."""
    y = np.zeros((B, T, C), dtype=np.float32)
    for c in range(N_CORES):
        b = c // HPC
        y[b] += np.asarray(results[c]["y"], dtype=np.float32)
    return y


def kernel(x, Wq, Wk, Wv, Wo):
    from concourse.bass_utils import run_bass_kernel_spmd

    nc = _get_compiled()
    in_maps = make_inputs(x, Wq, Wk, Wv, Wo)
    res = run_bass_kernel_spmd(nc, in_maps, list(range(N_CORES)))
    return assemble(res.results)
